# revision 1
# baseline (speedup 1.0000x reference)
"""Trainium2 Bass kernel for a 2-layer ResGatedGraphConv encoder.

Strategy (edge-parallel over 8 NeuronCores):
  - Nodes are permuted by degree rank and dealt round-robin to the 8 cores, so
    each core owns NPC nodes arranged in TPC tiles of 128 dst nodes whose
    degrees are nearly uniform within a tile.
  - Each edge lives on the core/tile/partition of its dst node; per tile the
    edge list of every dst node is padded to the tile max degree d_hat, so the
    per-edge gather lands token-major [128 dst, d_hat, 128] and aggregation is
    a plain free-axis reduction (no scatter, no indicator matmuls).
  - Per layer, a packed [q|v] node table [NT, 128] is built on device with PE
    matmuls and written to DRAM; messages gather rows of it with one indirect
    DMA per tile.  k is only needed per dst node: it is computed per tile
    [128, 64] and broadcast along the degree axis in the sigmoid argument.
  - h1 is exchanged between cores with an AllGather so layer 2 can build its
    node table from the full hidden state.

kernel(**inputs) takes the full (unsharded) inputs and returns the full
output; all sharding happens inside.
"""

import sys
import numpy as np

for _p in ("/opt/trn_rl_repo", "/opt/pypackages"):
    if _p not in sys.path:
        sys.path.append(_p)

N = 100000
E = 1600000
H = 64
NCORES = 8


class Cfg:
    def __init__(self, n, tpc):
        self.n = n
        self.tpc = tpc                      # dst tiles per core
        self.npc = tpc * 128                # nodes per core
        self.npad = NCORES * self.npc       # padded node count
        self.nt = self.npad + 128           # table rows (incl. dummy block)
        self.dummy = self.npad              # dummy (all-zero) table row
        assert self.npad >= n


FULL_CFG = Cfg(N, 98)


def host_prep(x, edge_index, cfg):
    """Permute nodes / build per-core gather schedules on the host."""
    n = cfg.n
    src = np.asarray(edge_index[0]).astype(np.int64)
    dst = np.asarray(edge_index[1]).astype(np.int64)
    deg = np.bincount(dst, minlength=n)

    # degree-rank round-robin: rank r -> core r%8, slot r//8
    rank_order = np.argsort(deg, kind="stable")  # node ids in degree order
    r = np.arange(n)
    node_core = np.empty(n, np.int64)
    node_slot = np.empty(n, np.int64)
    node_core[rank_order] = r % NCORES
    node_slot[rank_order] = r // NCORES
    tau = node_core * cfg.npc + node_slot      # table id of each node

    # per-edge position within its dst node's list
    order = np.argsort(dst, kind="stable")
    d_sorted = dst[order]
    first = np.searchsorted(d_sorted, np.arange(n))
    k_within = np.arange(len(dst)) - first[d_sorted]
    k_e = np.empty(len(dst), np.int64)
    k_e[order] = k_within

    e_core = node_core[dst]
    e_slot = node_slot[dst]
    e_tile = e_slot // 128
    e_part = e_slot % 128

    # per-tile max degree (shared across cores so programs are identical)
    deg_cs = np.zeros((NCORES, cfg.npc), np.int64)
    deg_cs[node_core, node_slot] = deg
    d_hat = deg_cs.reshape(NCORES, cfg.tpc, 128).max(axis=(0, 2))
    d_hat = np.maximum(d_hat, 1).astype(np.int64)
    off = np.concatenate([[0], np.cumsum(d_hat)])
    nblk = int(off[-1])

    tau_src = tau[src]
    gidx = np.full((NCORES, 128, nblk), cfg.dummy, np.int32)
    col = off[e_tile] + k_e
    gidx[e_core, e_part, col] = tau_src.astype(np.int32)

    # permuted feature table input, feature-major, zero padded
    xT_full = np.zeros((H, cfg.nt), np.float32)
    xT_full[:, tau] = np.asarray(x, np.float32).T

    return dict(
        gidx=gidx,
        d_hat=[int(v) for v in d_hat],
        off=[int(v) for v in off],
        nblk=nblk,
        tau=tau,
        xT_full=xT_full,
    )


def build_program(cfg, d_hat, nblk, table_dtype_bf16=False, debug_stage=None):
    import concourse.bass as bass
    import concourse.bacc as bacc
    import concourse.mybir as mybir
    import concourse.tile as tile
    from concourse.masks import make_identity

    f32 = mybir.dt.float32
    tdt = mybir.dt.bfloat16 if table_dtype_bf16 else f32
    tpc, npc, nt = cfg.tpc, cfg.npc, cfg.nt
    ntiles_table = nt // 128
    half = npc // 2  # packed resident layout: rows 0:64 first half, 64:128 second

    nc = bacc.Bacc("TRN2", target_bir_lowering=False, debug=False,
                   num_devices=NCORES)

    # ---- I/O ----
    xT_full = nc.dram_tensor("xT_full", [H, nt], f32, kind="ExternalInput")
    xT_own = nc.dram_tensor("xT_own", [H, npc], f32, kind="ExternalInput")
    gidx = nc.dram_tensor("gidx", [128, nblk], mybir.dt.int32,
                          kind="ExternalInput")
    wnames = {}
    for l in (1, 2):
        for w in ("Wqv", "Wk", "Ws", "Wl"):
            shape = [H, 128] if w == "Wqv" else [H, H]
            wnames[f"{w}{l}"] = nc.dram_tensor(f"{w}{l}", shape, f32,
                                               kind="ExternalInput")
        wnames[f"b{l}r"] = nc.dram_tensor(f"b{l}r", [1, H], f32,
                                          kind="ExternalInput")
        wnames[f"bl{l}c"] = nc.dram_tensor(f"bl{l}c", [H, 1], f32,
                                           kind="ExternalInput")
    out_shard = nc.dram_tensor("out_shard", [npc, H], f32,
                               kind="ExternalOutput")

    # ---- internal DRAM ----
    qv_t = [nc.dram_tensor(f"qv{l}_t", [nt, 128], tdt) for l in (1, 2)]
    h1_shard = nc.dram_tensor("h1_shard", [H, npc], f32)
    h1_gath = nc.dram_tensor("h1_gath", [NCORES * H, npc], f32,
                             addr_space="Shared")

    with tile.TileContext(nc) as tc:
        cp = tc.alloc_tile_pool(name="const", bufs=1)

        identity = cp.tile([128, 128], f32)
        make_identity(nc, identity[:])
        identity64 = cp.tile([64, 64], f32)
        make_identity(nc, identity64[:])
        ones_row = cp.tile([1, 128], f32)
        nc.vector.memset(ones_row[:], 1.0)

        gidx_sb = cp.tile([128, nblk], mybir.dt.int32)
        nc.sync.dma_start(out=gidx_sb[:], in_=gidx.ap()[:])

        wt = {}
        for l in (1, 2):
            for w in ("Wqv", "Wk", "Ws", "Wl"):
                shape = [H, 128] if w == "Wqv" else [H, H]
                wt[f"{w}{l}"] = cp.tile(shape, f32, name=f"{w}{l}", tag=f"{w}{l}")
                nc.sync.dma_start(out=wt[f"{w}{l}"][:],
                                  in_=wnames[f"{w}{l}"].ap()[:])
            wt[f"b{l}r"] = cp.tile([1, H], f32, name=f"b{l}r", tag=f"b{l}r")
            nc.sync.dma_start(out=wt[f"b{l}r"][:], in_=wnames[f"b{l}r"].ap()[:])
            wt[f"bl{l}c"] = cp.tile([H, 1], f32, name=f"bl{l}c", tag=f"bl{l}c")
            nc.sync.dma_start(out=wt[f"bl{l}c"][:],
                              in_=wnames[f"bl{l}c"].ap()[:])


        # ---------------- phase A: build [q|v] table ----------------
        def phase_a(layer):
            table = qv_t[layer - 1]
            wqv = wt[f"Wqv{layer}"]
            CH = 8
            with tc.tile_pool(name=f"pa{layer}", bufs=3) as pa, \
                 tc.tile_pool(name=f"pap{layer}", bufs=4, space="PSUM") as pap:
                for j0 in range(0, ntiles_table, CH):
                    ncnt = min(CH, ntiles_table - j0)
                    src_t = pa.tile([H, CH * 128], f32, tag="src")
                    if layer == 1:
                        nc.sync.dma_start(
                            out=src_t[:, 0:ncnt * 128],
                            in_=xT_full.ap()[:, j0 * 128:(j0 + ncnt) * 128])
                    else:
                        # source h1_gath with per-core reslicing; a chunk may
                        # cross core boundaries or run into the dummy tail
                        seg0 = 0
                        while seg0 < ncnt * 128:
                            g = j0 * 128 + seg0
                            if g >= cfg.npad:
                                nc.vector.memset(
                                    src_t[:, seg0:ncnt * 128], 0.0)
                                break
                            c = g // npc
                            n0 = g % npc
                            seglen = min(ncnt * 128 - seg0, npc - n0,
                                         cfg.npad - g)
                            nc.sync.dma_start(
                                out=src_t[:, seg0:seg0 + seglen],
                                in_=h1_gath.ap()[c * H:(c + 1) * H,
                                                 n0:n0 + seglen])
                            seg0 += seglen
                    st = pa.tile([128, CH * 128], tdt, tag="st")
                    for i in range(ncnt):
                        ps = pap.tile([128, 128], f32, tag="ps", space="PSUM")
                        nc.tensor.matmul(
                            ps[:], lhsT=src_t[:, i * 128:(i + 1) * 128],
                            rhs=wqv[:], start=True, stop=True)
                        nc.scalar.activation(
                            st[:, i * 128:(i + 1) * 128], ps[:],
                            mybir.ActivationFunctionType.Copy)
                    out_ap = table.ap()[j0 * 128:(j0 + ncnt) * 128, :]
                    out_ap = out_ap.rearrange("(c p) e -> p c e", p=128)
                    in_ap = st[:, 0:ncnt * 128].rearrange(
                        "p (c e) -> p c e", e=128)
                    nc.sync.dma_start(out=out_ap, in_=in_ap)

        # ---------------- phase B: gated conv + linear ----------------
        def conv_layer(layer, hsrc_dram, write_out=None):
            layer = 1 if hsrc_dram is xT_own else 2
            table = qv_t[layer - 1]
            wk, ws, wl = wt[f"Wk{layer}"], wt[f"Ws{layer}"], wt[f"Wl{layer}"]
            brow, blc = wt[f"b{layer}r"], wt[f"bl{layer}c"]
            with tc.tile_pool(name=f"pb{layer}", bufs=2) as pb, \
                 tc.tile_pool(name=f"pbs{layer}", bufs=3) as pbs, \
                 tc.tile_pool(name=f"pbp{layer}", bufs=2, space="PSUM") as pbp:
                off_c = 0
                for t in range(tpc):
                    dh = d_hat[t]
                    hot_t = pbs.tile([H, 128], f32, tag="hot", bufs=4)
                    nc.sync.dma_start(
                        out=hot_t[:],
                        in_=hsrc_dram.ap()[:, t * 128:(t + 1) * 128])
                    hot = hot_t[:]

                    # k tile for this dst tile: [128 n, 64 h]
                    kps = pbp.tile([128, H], f32, tag="kps", space="PSUM")
                    nc.tensor.matmul(kps[:], lhsT=hot, rhs=wk[:],
                                     start=True, stop=True)
                    ksb = pbs.tile([128, H], tdt, tag="ksb", bufs=4)
                    nc.scalar.activation(ksb[:], kps[:],
                                         mybir.ActivationFunctionType.Copy)

                    # gather q|v rows of all edges of this tile
                    qvg_f = pb.tile([128, dh * 128], tdt, tag="qvg", bufs=8)
                    for j in range(dh):
                        nc.gpsimd.indirect_dma_start(
                            out=qvg_f[:, j * 128:(j + 1) * 128],
                            out_offset=None,
                            in_=table.ap()[:, :],
                            in_offset=bass.IndirectOffsetOnAxis(
                                ap=gidx_sb[:, off_c + j:off_c + j + 1], axis=0),
                        )
                    qvg = qvg_f[:].rearrange("p (a b) -> p a b", b=128)

                    # sigarg = k[dst] + q ;  sig = sigmoid(sigarg)
                    sigarg = pbs.tile([128, dh, H], tdt, tag="sigarg", bufs=4)
                    kb = ksb[:].rearrange("p (o h) -> p o h", o=1)
                    kb = bass.AP(kb.tensor, kb.offset,
                                 [kb.ap[0], [0, dh], kb.ap[2]])
                    nc.vector.tensor_tensor(
                        out=sigarg[:], in0=qvg[:, :, 0:H], in1=kb,
                        op=mybir.AluOpType.add)
                    sig = pbs.tile([128, dh, H], mybir.dt.bfloat16, tag="sig", bufs=4)
                    nc.scalar.activation(
                        sig[:], sigarg[:],
                        mybir.ActivationFunctionType.Sigmoid)
                    # msg = sig * v
                    msg = pbs.tile([128, dh, H], mybir.dt.bfloat16, tag="msg", bufs=4)
                    nc.vector.tensor_tensor(
                        out=msg[:], in0=sig[:], in1=qvg[:, :, H:128],
                        op=mybir.AluOpType.mult)
                    # agg[p, h] = sum_k msg[p, k, h]
                    agg = pbs.tile([128, H], f32, tag="agg")
                    mt = msg[:].rearrange("p k h -> p h k")
                    nc.vector.tensor_reduce(
                        out=agg[:], in_=mt, axis=mybir.AxisListType.X,
                        op=mybir.AluOpType.add)

                    # conv out = agg + x@Ws + b  (token-major [128 n, 64 h])
                    cps = pbp.tile([128, H], f32, tag="cps", space="PSUM")
                    nc.tensor.matmul(cps[:], lhsT=hot, rhs=ws[:],
                                     start=True, stop=False)
                    nc.tensor.matmul(cps[:], lhsT=ones_row[:], rhs=brow[:],
                                     start=False, stop=True)
                    hc = pbs.tile([128, H], f32, tag="hc")
                    nc.vector.tensor_tensor(out=hc[:], in0=agg[:], in1=cps[:],
                                            op=mybir.AluOpType.add)

                    # transpose to feature-major, then linear + relu
                    tps = pbp.tile([H, 128], f32, tag="tps", space="PSUM")
                    nc.tensor.transpose(out=tps[:], in_=hc[:],
                                        identity=identity[:])
                    hcT = pbs.tile([H, 128], f32, tag="hcT")
                    nc.scalar.activation(hcT[:], tps[:],
                                         mybir.ActivationFunctionType.Copy)
                    lps = pbp.tile([H, 128], f32, tag="lps", space="PSUM")
                    nc.tensor.matmul(lps[:], lhsT=wl[:], rhs=hcT[:],
                                     start=True, stop=True)

                    if debug_stage in ("agg", "ksb", "hc", "qv0", "sg0", "ms0", "sg1", "ms1") and layer == 1:
                        dbg = {"agg": agg, "ksb": ksb, "hc": hc,
                               "qv0": qvg[:, 0, 0:H],
                               "sg0": sigarg[:, 0, :],
                               "ms0": msg[:, 0, :],
                               "sg1": sigarg[:, min(1, dh - 1), :],
                               "ms1": msg[:, min(1, dh - 1), :]}[debug_stage]
                        dstg = pbs.tile([128, H], f32, tag="dstg")
                        nc.vector.tensor_copy(dstg[:], dbg[:])
                        nc.sync.dma_start(
                            out=out_shard.ap()[t * 128:(t + 1) * 128, :],
                            in_=dstg[:])
                    if (layer == 1) if write_out is None else not write_out:
                        h1t = pbs.tile([H, 128], f32, tag="h2T")
                        nc.scalar.activation(
                            h1t[:], lps[:],
                            mybir.ActivationFunctionType.Relu,
                            bias=blc[:])
                        nc.sync.dma_start(
                            out=h1_shard.ap()[:, t * 128:(t + 1) * 128],
                            in_=h1t[:])
                    else:
                        h2T = pbs.tile([H, 128], f32, tag="h2T")
                        nc.scalar.activation(
                            h2T[:], lps[:],
                            mybir.ActivationFunctionType.Relu,
                            bias=blc[:])
                        ops = pbp.tile([128, H], f32, tag="kps", space="PSUM")
                        nc.tensor.transpose(out=ops[:], in_=h2T[:],
                                            identity=identity64[:])
                        osb = pbs.tile([128, H], f32, tag="osb")
                        nc.scalar.activation(osb[:], ops[:],
                                             mybir.ActivationFunctionType.Copy)
                        nc.sync.dma_start(
                            out=out_shard.ap()[t * 128:(t + 1) * 128, :],
                            in_=osb[:])
                    off_c += dh

        phase_a(1)
        tc.strict_bb_all_engine_barrier()
        conv_layer(1, xT_own, write_out=(True if debug_stage == "h1" else None))

        # exchange h1 across cores
        if debug_stage is None:
            nc.gpsimd.collective_compute(
                "AllGather",
                mybir.AluOpType.bypass,
                replica_groups=[list(range(NCORES))],
                ins=[h1_shard.ap()[:, :]],
                outs=[h1_gath.ap()[:, :]],
            )

            phase_a(2)
            tc.strict_bb_all_engine_barrier()
            conv_layer(2, h1_shard)

        cp.release()

    nc.compile()
    return nc


def _pack_inputs(prep, inputs, cfg):
    """Build the 8 per-core input maps."""
    xT_full = prep["xT_full"]
    base = {"xT_full": xT_full}
    for l, (wq, wv, wk, ws, b, wl, bl) in {
        1: ("Wq1", "Wv1", "Wk1", "Ws1", "b1", "Wl1", "bl1"),
        2: ("Wq2", "Wv2", "Wk2", "Ws2", "b2", "Wl2", "bl2"),
    }.items():
        base[f"Wqv{l}"] = np.ascontiguousarray(
            np.concatenate([np.asarray(inputs[wq], np.float32),
                            np.asarray(inputs[wv], np.float32)], axis=1))
        base[f"Wk{l}"] = np.ascontiguousarray(np.asarray(inputs[wk], np.float32))
        base[f"Ws{l}"] = np.ascontiguousarray(np.asarray(inputs[ws], np.float32))
        base[f"Wl{l}"] = np.ascontiguousarray(np.asarray(inputs[wl], np.float32))
        base[f"b{l}r"] = np.asarray(inputs[b], np.float32).reshape(1, H)
        base[f"bl{l}c"] = np.asarray(inputs[bl], np.float32).reshape(H, 1)

    in_maps = []
    for c in range(NCORES):
        m = dict(base)
        m["xT_own"] = np.ascontiguousarray(
            xT_full[:, c * cfg.npc:(c + 1) * cfg.npc])
        m["gidx"] = np.ascontiguousarray(prep["gidx"][c])
        in_maps.append(m)
    return in_maps


def run(inputs, cfg=FULL_CFG, sim=False, trace=False, table_bf16=True,
        debug_stage=None):
    from concourse import bass_utils

    x = np.asarray(inputs["x"], np.float32)
    prep = host_prep(x, inputs["edge_index"], cfg)
    nc = build_program(cfg, prep["d_hat"], prep["nblk"],
                       table_dtype_bf16=table_bf16, debug_stage=debug_stage)
    in_maps = _pack_inputs(prep, inputs, cfg)

    if sim:
        from concourse.bass_interp import MultiCoreSim
        ms = MultiCoreSim(nc, num_cores=NCORES, trace=False)
        for c in range(NCORES):
            for name, arr in in_maps[c].items():
                ms.cores[c].tensor(name)[:] = arr
        ms.simulate(check_with_hw=False)
        shards = [np.array(ms.cores[c].tensor("out_shard")) for c in
                  range(NCORES)]
        res = None
    else:
        if trace:
            try:
                sys.path.insert(0, "/root/problem")
                import ntff_hook  # noqa: F401
            except Exception:
                trace = False
        res = bass_utils.run_bass_kernel_spmd(
            nc, in_maps, core_ids=list(range(NCORES)), trace=trace)
        shards = [res.results[c]["out_shard"] for c in range(NCORES)]

    full = np.concatenate(shards, axis=0)     # [npad, H] in permuted order
    out = np.empty((cfg.n, H), np.float32)
    out[:, :] = full[prep["tau"], :]
    return out, res


def kernel(**inputs):
    out, _ = run(inputs, FULL_CFG, sim=False, trace=False)
    return out.astype(np.float32)

